# revision 12
# baseline (speedup 1.0000x reference)
"""Trainium2 Bass kernel for a local-window multi-head attention block.

Math (per batch element b, all in one NeuronCore; batch is data-parallel
across the 8 cores):
    qkv  = x @ w_qkv.T                      [N, 2304]
    q,k,v split into 12 heads of dim 64, q scaled by 1/8
    S    = q @ k.T + local mask             (mask: |dh|<=3, |dw|<=5 on a 16x64 grid)
    P    = softmax(S); O = P @ v
    out  = O @ w_proj.T + b_proj

Device layout notes:
  - Everything is computed transposed (channels on partitions):
    qkvT = w_qkv @ x.T via PE, S^T tiles per 128-token m-chunk over a
    512-wide sliding n-window (the local mask is fully contained in that
    band), softmax without max-subtraction (scores are tiny), row sums via
    an appended ones-column in the P@V matmul, 1/r via a fast DVE
    reciprocal and a DRAM-bounce partition broadcast.
  - QKV + proj matmuls run in float32r (full PE rate at free-dim >= 256),
    attention internals (q,k,exp(S),v) in bf16.
"""

import os
import sys

sys.path.insert(0, "/opt/trn_rl_repo")

import numpy as np

B, N, DIM = 8, 1024, 768
NH, HD = 12, 64
SCALE = HD ** -0.5
P = 128
CN = DIM // P            # 6 channel chunks
MC = N // P              # 8 token chunks (m side)
WIN = 512                # attention band width
NT = N // WIN            # 2 n-tiles
WIN_START = [64 * min(max(2 * c - 3, 0), 8) for c in range(MC)]

_PROG = None


def _emit(ctx, tc, aps, debug=None):
    import concourse.bass as bass
    import concourse.mybir as mybir

    nc = tc.nc
    f32 = mybir.dt.float32
    bf16 = mybir.dt.bfloat16
    f32r = mybir.dt.float32r
    AF = mybir.ActivationFunctionType

    xT, wqkvT, wprojT, biasT, bmask, outT = aps

    consts = ctx.enter_context(tc.tile_pool(name="consts", bufs=1))

    xT_sb = consts.tile([P, CN, N], f32r)
    nc.sync.dma_start(xT_sb[:], xT.rearrange("(co p) n -> p co n", p=P))
    wprojT_sb = consts.tile([P, CN, DIM], f32r)
    nc.sync.dma_start(wprojT_sb[:], wprojT.rearrange("(co p) o -> p co o", p=P))
    bias_sb = consts.tile([P, CN], f32)
    nc.sync.dma_start(bias_sb[:], biasT[:])
    bmask_sb = consts.tile([P, MC, WIN], bf16)
    nc.sync.dma_start(bmask_sb[:], bmask[:])

    qkT_sb = consts.tile([P, 2 * CN, N], bf16)     # chunks 0..5 = q, 6..11 = k
    V_sb = consts.tile([P, MC, NH, 66], bf16)      # col 64 = ones, 65 = pad
    OT_sb = consts.tile([P, CN, N], f32r)
    nc.vector.memset(V_sb[:, :, :, 64:65], 1.0)
    if debug is not None:
        debug.update(qkT_sb=qkT_sb, V_sb=V_sb, OT_sb=OT_sb)

    # ---------------- QKV ----------------
    with tc.tile_pool(name="wstream", bufs=3) as wpool, \
         tc.tile_pool(name="qkvpsum", bufs=4, space="PSUM") as qpsum:
        # V part first: psum[n-chunk, o] with n on partitions.
        for oh in range(2):
            wv = wpool.tile([P, CN, 384], f32r, name="wv", tag="wv")
            nc.sync.dma_start(
                wv[:],
                wqkvT[:, 2 * DIM + 384 * oh: 2 * DIM + 384 * (oh + 1)]
                .rearrange("(co p) o -> p co o", p=P),
            )
            for m in range(MC):
                vps = qpsum.tile([P, 384], f32, name="vps", tag="vps")
                for k in range(CN):
                    nc.tensor.matmul(
                        vps[:],
                        lhsT=xT_sb[:, k, P * m: P * (m + 1)],
                        rhs=wv[:, k, :],
                        start=(k == 0),
                        stop=(k == CN - 1),
                    )
                nc.scalar.activation(
                    V_sb[:, m, 6 * oh: 6 * (oh + 1), 0:64],
                    vps[:].rearrange("p (a b) -> p a b", b=64),
                    AF.Copy,
                )
        # Q/K part: psum[o-chunk, n] with o on partitions (i.e. transposed).
        for hp in range(CN):
            for qk in range(2):
                col = DIM * qk + P * hp
                w = wpool.tile([P, CN, P], f32r, name="w", tag="wqk")
                nc.sync.dma_start(
                    w[:], wqkvT[:, col: col + P].rearrange("(co p) o -> p co o", p=P)
                )
                for t in range(NT):
                    qps = qpsum.tile([P, WIN], f32, name="qps", tag="qkps")
                    for k in range(CN):
                        nc.tensor.matmul(
                            qps[:],
                            lhsT=w[:, k, :],
                            rhs=xT_sb[:, k, WIN * t: WIN * (t + 1)],
                            start=(k == 0),
                            stop=(k == CN - 1),
                        )
                    nc.vector.tensor_copy(
                        qkT_sb[:, CN * qk + hp, WIN * t: WIN * (t + 1)], qps[:]
                    )

    # ---------------- attention ----------------
    with tc.tile_pool(name="spsum", bufs=2, space="PSUM") as spool, \
         tc.tile_pool(name="otpsum", bufs=3, space="PSUM") as opool, \
         tc.tile_pool(name="etp", bufs=3) as etpool, \
         tc.tile_pool(name="rp", bufs=4) as rpool, \
         tc.tile_pool(name="invbp", bufs=3) as bpool, \
         tc.tile_pool(name="stp", bufs=2) as stpool:
        for hp in range(CN):
            for hh in range(2):
                h = 2 * hp + hh
                prange = slice(64 * hh, 64 * hh + 64)
                et = etpool.tile([P, MC, WIN], bf16, name="et", tag="et")
                for cp in range(MC // 2):
                    sps = spool.tile([P, 2 * WIN], f32, name="sps", tag="sps")
                    for j in range(2):
                        c = 2 * cp + j
                        s = WIN_START[c]
                        nc.tensor.matmul(
                            sps[:, WIN * j: WIN * (j + 1)],
                            lhsT=qkT_sb[prange, CN + hp, P * c: P * (c + 1)],
                            rhs=qkT_sb[prange, hp, s: s + WIN],
                            start=True,
                            stop=True,
                            tile_position=(64 * hh, 0),
                        )
                    nc.scalar.activation(
                        et[:, 2 * cp: 2 * cp + 2, :].rearrange("p a b -> p (a b)"),
                        sps[:],
                        AF.Exp,
                    )
                    nc.vector.tensor_mul(
                        et[:, 2 * cp: 2 * cp + 2, :].rearrange("p a b -> p (a b)"),
                        et[:, 2 * cp: 2 * cp + 2, :].rearrange("p a b -> p (a b)"),
                        bmask_sb[:, 2 * cp: 2 * cp + 2, :].rearrange("p a b -> p (a b)"),
                    )
                if debug is not None and ("d_et%d" % h) in debug:
                    nc.sync.dma_start(debug["d_et%d" % h][:], et[:])
                # P@V with ones column: psum rows 0:64 = O^T, row 64 = sum(E).
                for t in range(NT):
                    ot = opool.tile([P, WIN], f32, name="ot", tag="ot")
                    cs = [c for c in range(MC)
                          if min(WIN_START[c] + WIN, WIN * (t + 1)) > max(WIN_START[c], WIN * t)]
                    # widest-overlap chunk first so the start=True matmul
                    # covers the whole psum range (per-element has_written
                    # then only ever accumulates into written elements)
                    cs.sort(key=lambda c: max(WIN_START[c], WIN * t)
                            - min(WIN_START[c] + WIN, WIN * (t + 1)))
                    for i, c in enumerate(cs):
                        lo = max(WIN_START[c], WIN * t)
                        hi = min(WIN_START[c] + WIN, WIN * (t + 1))
                        nc.tensor.matmul(
                            ot[0:65, lo - WIN * t: hi - WIN * t],
                            lhsT=V_sb[:, c, h, 0:65],
                            rhs=et[:, c, lo - WIN_START[c]: hi - WIN_START[c]],
                            start=(i == 0),
                            stop=(i == len(cs) - 1),
                        )
                    # reciprocal_approx_fast misreads PSUM operands on HW;
                    # stage the r-row through SBUF first
                    rrow = rpool.tile([1, WIN], f32, name="rrow", tag="rrow")
                    nc.scalar.activation(rrow[:], ot[64:65, :], AF.Copy)
                    invrow = rpool.tile([1, WIN], f32, name="invrow", tag="invrow")
                    nc.vector.reciprocal_approx_fast(invrow[:], rrow[:])
                    invb = bpool.tile([64, WIN], f32, name="invb", tag="invb")
                    nc.gpsimd.partition_broadcast(invb[:], invrow[:])
                    if hh == 0:
                        nc.vector.tensor_mul(
                            OT_sb[0:64, hp, WIN * t: WIN * (t + 1)],
                            ot[0:64, :], invb[:],
                        )
                    else:
                        st = stpool.tile([64, WIN], f32r, name="st", tag="st")
                        nc.vector.tensor_mul(st[:], ot[0:64, :], invb[:])
                        nc.sync.dma_start(
                            OT_sb[64:128, hp, WIN * t: WIN * (t + 1)], st[:]
                        )

    # ---------------- output projection ----------------
    with tc.tile_pool(name="pjpsum", bufs=4, space="PSUM") as ppool, \
         tc.tile_pool(name="outst", bufs=4) as ostpool:
        for oc in range(CN):
            for t in range(NT):
                pps = ppool.tile([P, WIN], f32, name="pps", tag="pj")
                for k in range(CN):
                    nc.tensor.matmul(
                        pps[:],
                        lhsT=wprojT_sb[:, k, P * oc: P * (oc + 1)],
                        rhs=OT_sb[:, k, WIN * t: WIN * (t + 1)],
                        start=(k == 0),
                        stop=(k == CN - 1),
                    )
                ost = ostpool.tile([P, WIN], f32, name="ost", tag="ost")
                nc.scalar.activation(ost[:], pps[:], AF.Identity,
                                     bias=bias_sb[:, oc: oc + 1])
                nc.sync.dma_start(
                    outT[P * oc: P * (oc + 1), WIN * t: WIN * (t + 1)], ost[:]
                )


def _build():
    global _PROG
    if _PROG is not None:
        return _PROG
    from contextlib import ExitStack

    from concourse import bacc
    import concourse.mybir as mybir
    import concourse.tile as tile

    f32 = mybir.dt.float32
    bf16 = mybir.dt.bfloat16
    f32r = mybir.dt.float32r

    nc = bacc.Bacc("TRN2", target_bir_lowering=False, debug=False,
                   enable_asserts=False)
    xT = nc.dram_tensor("xT", [DIM, N], f32r, kind="ExternalInput").ap()
    wqkvT = nc.dram_tensor("wqkvT", [DIM, 3 * DIM], f32r, kind="ExternalInput").ap()
    wprojT = nc.dram_tensor("wprojT", [DIM, DIM], f32r, kind="ExternalInput").ap()
    biasT = nc.dram_tensor("biasT", [P, CN], f32, kind="ExternalInput").ap()
    bmask = nc.dram_tensor("bmask", [P, MC, WIN], bf16, kind="ExternalInput").ap()
    outT = nc.dram_tensor("outT", [DIM, N], f32, kind="ExternalOutput").ap()

    with tile.TileContext(nc) as tc:
        with ExitStack() as ctx:
            _emit(ctx, tc, (xT, wqkvT, wprojT, biasT, bmask, outT))
    nc.compile()
    _PROG = nc
    return nc


def _host_inputs(x, w_qkv, w_proj, b_proj, mask):
    import ml_dtypes

    x = np.asarray(x, dtype=np.float32)
    w_qkv = np.asarray(w_qkv, dtype=np.float32)
    w_proj = np.asarray(w_proj, dtype=np.float32)
    b_proj = np.asarray(b_proj, dtype=np.float32)
    mask = np.asarray(mask, dtype=np.float32)

    wq = w_qkv.copy()
    wq[0:DIM] *= SCALE
    wqkvT = np.ascontiguousarray(wq.T)                       # [768, 2304]
    wprojT = np.ascontiguousarray(w_proj.T)                  # [768, 768]
    biasT = np.ascontiguousarray(b_proj.reshape(CN, P).T)    # [128, 6]

    vis = (mask[0, 0] == 0.0)
    bm = np.zeros((P, MC, WIN), dtype=ml_dtypes.bfloat16)
    for c in range(MC):
        s = WIN_START[c]
        bm[:, c, :] = vis[c * P:(c + 1) * P, s: s + WIN]

    in_maps = []
    for b in range(B):
        in_maps.append({
            "xT": np.ascontiguousarray(x[b].T),
            "wqkvT": wqkvT,
            "wprojT": wprojT,
            "biasT": biasT,
            "bmask": bm,
        })
    return in_maps


PROFILE = False
LAST_RESULT = None


def kernel(x, w_qkv, w_proj, b_proj, mask):
    global LAST_RESULT
    from concourse.bass_utils import run_bass_kernel_spmd

    nc = _build()
    in_maps = _host_inputs(x, w_qkv, w_proj, b_proj, mask)
    res = run_bass_kernel_spmd(nc, in_maps, core_ids=list(range(B)),
                               trace=PROFILE)
    LAST_RESULT = res
    out = np.stack([np.asarray(res.results[b]["outT"]).T for b in range(B)])
    return np.ascontiguousarray(out.astype(np.float32))
